# revision 31
# baseline (speedup 1.0000x reference)
"""Chunk-causal Whisper attention (B=4, T=1500, D=1024, H=16) on 8 NeuronCores.

Sharding: core c = (batch b = c//2, head-half hh = c%2). Each core runs one
batch element with 8 of the 16 heads (512 of 1024 channels). All on-chip
tensors are kept transposed: scoresT[k,q] = K @ Q^T per head, so the softmax
reduction runs along the partition (key) axis. A ones-column appended to V
makes the PV matmul produce both the unnormalized output and the softmax
denominator; normalization broadcasts 1/denom across partitions with a gpsimd
partition_broadcast and one DVE multiply. The Wo matmul consumes the
transposed attention output directly; the host sums the two head-half
partials and adds the constant (bv @ Wo + bo) (bv folds through softmax
because the probabilities sum to 1).

All matmul operands are bf16 (PSUM accumulation stays fp32). Score tiles are
exp'd in PAIRS (one scalar-engine activation over two adjacent PSUM banks) to
amortize the activation engine's access latency; masking is a multiplicative
{0,1} bf16 mask applied to exp(scores) on the vector engine (the additive
-inf form is equivalent since exp(s+m) = exp(s)*exp(m)).

Schedule: a minimal phase A (K tb0/tb1 cc0, Q tb0 cc0, V tk0-4) unblocks
attention early; the remaining Q/K/V projection units and the per-qc output
projections are interleaved between attention groups so the tensor engine
stays fed while attention alone would be activation-bound (qc1/qc2), and
attention pairs are software-pipelined with lag 1 so PV matmuls never block
the scores stream on the exp latency. Output-projection results stream to
DRAM per (oc, qc) tile.

T is padded 1500 -> 1536 with zeros (hsT columns), so every tile is a clean
128/512 multiple. Padded keys contribute nothing (V rows and the ones-column
are zero there); padded query columns are finite garbage and dropped on the
host. The chunk-causal mask (chunk(j) <= chunk(i) or j <= i+50 with
chunk=100-frame blocks) is monotone per query, so each (128-key x 512-query)
block is fully-allowed, fully-masked (skipped), or partial.
"""

import sys

import numpy as np
import ml_dtypes

if "/opt/trn_rl_repo" not in sys.path:
    sys.path.insert(0, "/opt/trn_rl_repo")

import concourse.tile as tile  # noqa: E402
from concourse import bacc, mybir  # noqa: E402
import concourse.bass_utils as bass_utils  # noqa: E402

B, T, D = 4, 1500, 1024
H, HD = 16, 64
CHUNK, LOOK = 100, 50
TP = 1536          # padded sequence length
CH = 512           # channels per core (8 heads)
HL = 8             # heads per core
NCORES = 8
SCALE = HD ** -0.5
NEG = -30.0
QB = 512           # query block (matmul moving free dim)
KB = 128           # key block (contraction tile)
NQC = TP // QB     # 3
NKC = TP // KB     # 12
NDC = D // 128     # 8
NCC = CH // 128    # 4
NOC = D // 128     # 8 output-column chunks
F32 = mybir.dt.float32
BF16 = mybir.dt.bfloat16


def _jmax(i):
    """Largest key index query i may attend to."""
    return max((i // CHUNK) * CHUNK + CHUNK - 1, i + LOOK)


def _classify():
    """Per (qc, kc) block: 'skip' | 'full' | index into the mask tensor."""
    status = {}
    masks = []
    for qc in range(NQC):
        q0 = qc * QB
        hi = max(_jmax(i) for i in range(q0, min(q0 + QB, T)))
        lo = _jmax(q0)
        for kc in range(NKC):
            k0 = kc * KB
            if k0 > hi:
                status[(qc, kc)] = "skip"
                continue
            if k0 + KB - 1 <= lo and k0 + KB <= T:
                status[(qc, kc)] = "full"
                continue
            # multiplicative {0,1} mask applied to exp(scores)
            m = np.zeros((KB, QB), np.float32)
            for ii in range(QB):
                i = min(q0 + ii, T - 1)  # padded queries reuse the last row
                n_ok = min(min(_jmax(i), T - 1) + 1 - k0, KB)
                if n_ok > 0:
                    m[:n_ok, ii] = 1.0
            status[(qc, kc)] = len(masks)
            masks.append(m)
    return status, masks


_STATUS, _MASKS = _classify()
NPART = len(_MASKS)

# Vx ones-column values: 1.0 for real keys, 0.0 for the padded tail.
_VONES = np.zeros((128, NKC, HL), np.float32)
for _tk in range(NKC):
    for _p in range(128):
        if _tk * KB + _p < T:
            _VONES[_p, _tk, :] = 1.0


def _build():
    nc = bacc.Bacc("TRN2", target_bir_lowering=False, debug=False)
    hsT = nc.dram_tensor("hsT", [D, TP], BF16, kind="ExternalInput")[:]
    wq = nc.dram_tensor("wq", [D, CH], BF16, kind="ExternalInput")[:]
    wk = nc.dram_tensor("wk", [D, CH], BF16, kind="ExternalInput")[:]
    wv = nc.dram_tensor("wv", [D, CH], BF16, kind="ExternalInput")[:]
    bqs = nc.dram_tensor("bqs", [CH], F32, kind="ExternalInput")[:]
    wo = nc.dram_tensor("wo", [CH, D], BF16, kind="ExternalInput")[:]
    maskT = nc.dram_tensor("maskT", [NPART, KB, QB], BF16, kind="ExternalInput")[:]
    vones = nc.dram_tensor("vones", [128, NKC, HL], BF16, kind="ExternalInput")[:]
    outT = nc.dram_tensor("outT", [D, TP], F32, kind="ExternalOutput")[:]

    hsT_r = hsT.rearrange("(a p) t -> a p t", p=128)
    wq_r = wq.rearrange("(a p) c -> a p c", p=128)
    wk_r = wk.rearrange("(a p) c -> a p c", p=128)
    wv_r = wv.rearrange("(a p) c -> a p c", p=128)
    wo_r = wo.rearrange("(a p) o -> a p o", p=128)
    outT_r = outT.rearrange("(a p) t -> a p t", p=128)

    ExpF = mybir.ActivationFunctionType.Exp

    with tile.TileContext(nc) as tc, \
         tc.tile_pool(name="per", bufs=1) as per, \
         tc.tile_pool(name="ex", bufs=8) as ep, \
         tc.tile_pool(name="sm", bufs=6) as sp, \
         tc.tile_pool(name="fin", bufs=3) as fp, \
         tc.tile_pool(name="ps_p", bufs=2, space="PSUM") as ps_p, \
         tc.tile_pool(name="ps_s", bufs=2, space="PSUM") as ps_s, \
         tc.tile_pool(name="ps_pv", bufs=2, space="PSUM") as ps_pv:
        KT = per.tile([128, NCC, TP], BF16)        # K^T: [c, cc, t]
        QT = per.tile([128, NCC, TP], BF16)        # Q^T (scale+bias folded)
        Vx = per.tile([128, NKC, HL, HD + 1], BF16)  # V + ones column
        AT = per.tile([128, NCC, TP], BF16)        # normalized attnT
        hs_sb = per.tile([128, NDC, TP], BF16)
        wk_sb = per.tile([128, NDC, CH], BF16)
        wq_sb = per.tile([128, NDC, CH], BF16)
        wv_sb = per.tile([128, NDC, CH], BF16)
        wo_sb = per.tile([128, NCC, D], BF16)
        mk_sb = per.tile([128, NPART, QB], BF16)
        bq_sb = per.tile([128, NCC], F32)

        # DMA priority: feed the first projection chains (hs+wk) before
        # anything else; weights/masks needed later come later.
        # vones must land before any V-projection copy touches Vx: its strided
        # 2-byte column writes race with concurrent engine writes to adjacent
        # bytes of the same tile if the DMA is deferred.
        nc.sync.dma_start(bq_sb[:], bqs.rearrange("(a p) -> p a", p=128))
        nc.sync.dma_start(Vx[:, :, :, HD:HD + 1], vones[:, :, :, None])
        for dc in range(NDC):
            nc.sync.dma_start(hs_sb[:, dc, :], hsT_r[dc])
            nc.sync.dma_start(wk_sb[:, dc, :], wk_r[dc])
        for dc in range(NDC):
            nc.sync.dma_start(wq_sb[:, dc, :], wq_r[dc])
            nc.sync.dma_start(wv_sb[:, dc, :], wv_r[dc])
        nc.sync.dma_start(mk_sb[:], maskT.rearrange("n p q -> p n q"))
        for cc in range(NCC):
            nc.sync.dma_start(wo_sb[:, cc, :], wo_r[cc])

        # ---- projection unit emitters (one PSUM bank each) ----
        def emit_k(tb, cc, act, t0=0, t1=QB):
            ts = slice(tb * QB + t0, tb * QB + t1)
            ps = ps_p.tile([128, QB], F32, tag="p", name=f"k{tb}{cc}")
            for dc in range(NDC):
                nc.tensor.matmul(
                    ps[:, 0:t1 - t0], wk_sb[:, dc, cc * 128:(cc + 1) * 128],
                    hs_sb[:, dc, ts], start=(dc == 0), stop=(dc == NDC - 1))
            if act:
                nc.scalar.copy(KT[:, cc, ts], ps[:, 0:t1 - t0])
            else:
                nc.vector.tensor_copy(KT[:, cc, ts], ps[:, 0:t1 - t0])

        def emit_q(tb, cc, act):
            ts = slice(tb * QB, (tb + 1) * QB)
            ps = ps_p.tile([128, QB], F32, tag="p", name=f"q{tb}{cc}")
            for dc in range(NDC):
                nc.tensor.matmul(
                    ps[:], wq_sb[:, dc, cc * 128:(cc + 1) * 128],
                    hs_sb[:, dc, ts], start=(dc == 0), stop=(dc == NDC - 1))
            if act:
                nc.scalar.add(QT[:, cc, ts], ps[:], bq_sb[:, cc:cc + 1])
            else:
                nc.vector.tensor_scalar_add(
                    QT[:, cc, ts], ps[:], bq_sb[:, cc:cc + 1])

        def emit_v(tk, act):
            ps = ps_p.tile([128, CH], F32, tag="p", name=f"v{tk}")
            for dc in range(NDC):
                nc.tensor.matmul(
                    ps[:], hs_sb[:, dc, tk * KB:(tk + 1) * KB],
                    wv_sb[:, dc, :], start=(dc == 0), stop=(dc == NDC - 1))
            dst = Vx[:, tk, :, 0:HD]
            src = ps[:].rearrange("p (h d) -> p h d", d=HD)
            if act:
                nc.scalar.copy(dst, src)
            else:
                nc.vector.tensor_copy(dst, src)

        # ---- attention group: software-pipelined pairs (lag 1) ----
        def attn(h, qc):
            pb = 64 * (h % 2)
            cc = h // 2
            qs = slice(qc * QB, (qc + 1) * QB)
            kcs = [kc for kc in range(NKC) if _STATUS[(qc, kc)] != "skip"]
            pairs = [kcs[i:i + 2] for i in range(0, len(kcs), 2)]
            nk = len(kcs)
            pv = ps_pv.tile([HD + 1, QB], F32)
            state = [0]

            def emit_pv(pair_kcs, ex):
                for j, kc in enumerate(pair_kcs):
                    nc.tensor.matmul(
                        pv[:], Vx[:, kc, h, :], ex[:, j, :],
                        start=(state[0] == 0), stop=(state[0] == nk - 1))
                    state[0] += 1

            # two pairs per pipeline step: scores batch in fours before the
            # previous step's PV batch, halving PE weight-shape switches and
            # doubling the exp->PV lag slack.
            pending = []
            for i in range(0, len(pairs), 3):
                chunk = pairs[i:i + 3]
                staged = []
                for pair in chunk:
                    ss = ps_s.tile([128, 2, QB], F32)
                    for j, kc in enumerate(pair):
                        nc.tensor.matmul(
                            ss[:, j, :],
                            KT[pb:pb + 64, cc, kc * KB:(kc + 1) * KB],
                            QT[pb:pb + 64, cc, qs], start=True, stop=True)
                    ex = ep.tile([128, 2, QB], BF16, tag="e", name="e")
                    if len(pair) == 2:
                        nc.scalar.activation(ex[:], ss[:], ExpF)
                    else:
                        nc.scalar.activation(ex[:, 0, :], ss[:, 0, :], ExpF)
                    sts = [_STATUS[(qc, kc)] for kc in pair]
                    if (len(pair) == 2 and sts[0] != "full"
                            and sts[1] != "full" and sts[1] == sts[0] + 1):
                        nc.vector.tensor_mul(
                            ex[:], ex[:], mk_sb[:, sts[0]:sts[0] + 2, :])
                    else:
                        for j, st in enumerate(sts):
                            if st != "full":
                                nc.vector.tensor_mul(
                                    ex[:, j, :], ex[:, j, :], mk_sb[:, st, :])
                    staged.append((pair, ex))
                for pk, e in pending:
                    emit_pv(pk, e)
                pending = staged
            for pk, e in pending:
                emit_pv(pk, e)
            # normalize: 1/denominator broadcast across the head partitions
            dn = sp.tile([1, QB], F32, tag="dn", name="dn")
            nc.vector.tensor_copy(dn[:], pv[HD:HD + 1, :])
            rc = sp.tile([1, QB], F32, tag="recip", name="recip")
            nc.vector.reciprocal_approx_fast(rc[:], dn[:])
            bc = sp.tile([HD, QB], F32, tag="bcast", name="bcast")
            nc.gpsimd.partition_broadcast(bc[:], rc[:])
            nc.vector.tensor_mul(AT[pb:pb + 64, cc, qs], pv[0:HD, :], bc[:])

        def phase3(qc, ocs=range(NOC)):
            qs = slice(qc * QB, (qc + 1) * QB)
            for oc in ocs:
                po = ps_p.tile([128, QB], F32, tag="p", name=f"o{oc}")
                for ccc in range(NCC):
                    nc.tensor.matmul(
                        po[:], wo_sb[:, ccc, oc * 128:(oc + 1) * 128],
                        AT[:, ccc, qs], start=(ccc == 0), stop=(ccc == NCC - 1))
                fin = fp.tile([128, QB], F32, tag="fin", name="fin")
                nc.vector.tensor_copy(fin[:], po[:])
                nc.sync.dma_start(outT_r[oc][:, qs], fin[:])

        # ---- phase A: minimal head so attention can start early ----
        # K/Q epilogues on ACT, V copies on DVE, so ACT is free for the
        # first score exps.
        emit_k(0, 0, True)
        emit_k(1, 0, True)
        emit_q(0, 0, True)
        for tk in range(5):
            emit_v(tk, False)

        # Projections interleaved between attention groups (epilogues on
        # DVE).  Late-needed units (V9-11, K tb2 tail, Q tb2) are pushed
        # into the qc1 region, where attention alone leaves the tensor
        # engine idle (ACT-bound); k8 = the kc8 slice of K tb2 that qc1
        # itself needs.
        ILV = {
            (0, 0): [("k", 0, 1), ("k", 1, 1), ("q", 0, 1)],
            (0, 1): [("k", 0, 2), ("k", 1, 2), ("q", 0, 2)],
            (0, 2): [("k", 0, 3), ("k", 1, 3), ("q", 0, 3)],
            (0, 3): [("q", 1, 0), ("q", 1, 1)],
            (0, 4): [("q", 1, 2), ("q", 1, 3)],
            (0, 5): [("v", 5, None), ("v", 6, None)],
            (0, 6): [("v", 7, None), ("v", 8, None)],
            (0, 7): [("k8", 2, 0), ("k8", 2, 1), ("k8", 2, 2), ("k8", 2, 3)],
            (1, 0): [("v", 9, None), ("kr", 2, 0)],
            (1, 1): [("v", 10, None), ("kr", 2, 1)],
            (1, 2): [("v", 11, None), ("kr", 2, 2)],
            (1, 3): [("kr", 2, 3), ("q", 2, 0), ("p3", 0, (0, 2))],
            (1, 4): [("q", 2, 1), ("p3", 0, (2, 4))],
            (1, 5): [("q", 2, 2), ("p3", 0, (4, 6))],
            (1, 6): [("q", 2, 3), ("p3", 0, (6, 8))],
            (2, 0): [("p3", 1, (0, 2))],
            (2, 1): [("p3", 1, (2, 4))],
            (2, 2): [("p3", 1, (4, 6))],
            (2, 3): [("p3", 1, (6, 8))],
        }

        for qc in range(NQC):
            for h in range(HL):
                attn(h, qc)
                for kind, a, b in ILV.get((qc, h), []):
                    if kind == "k":
                        emit_k(a, b, False)
                    elif kind == "k8":
                        emit_k(a, b, False, t0=0, t1=KB)
                    elif kind == "kr":
                        emit_k(a, b, False, t0=KB, t1=QB)
                    elif kind == "q":
                        emit_q(a, b, False)
                    elif kind == "p3":
                        phase3(a, range(*b))
                    else:
                        emit_v(a, False)
            if qc == NQC - 1:
                phase3(qc)

    nc.finalize()
    return nc


_NC = None


def _get_nc():
    global _NC
    if _NC is None:
        _NC = _build()
    return _NC


def _make_in_maps(hidden_states, Wq, bq, Wk, Wv, Wo):
    hs = np.ascontiguousarray(hidden_states, np.float32)
    Wq = np.asarray(Wq, np.float32)
    Wk = np.asarray(Wk, np.float32)
    Wv = np.asarray(Wv, np.float32)
    Wo = np.asarray(Wo, np.float32)
    bq = np.asarray(bq, np.float32)

    bf = ml_dtypes.bfloat16
    mask_arr = np.ascontiguousarray(np.stack(_MASKS)).astype(bf)
    wq_s = Wq * np.float32(SCALE)
    vones_bf = _VONES.astype(bf)

    in_maps = []
    for core in range(NCORES):
        b, hh = core // 2, core % 2
        sl = slice(hh * CH, (hh + 1) * CH)
        hsT_pad = np.zeros((D, TP), np.float32)
        hsT_pad[:, :T] = hs[b].T
        in_maps.append({
            "hsT": hsT_pad.astype(bf),
            "wq": np.ascontiguousarray(wq_s[:, sl]).astype(bf),
            "wk": np.ascontiguousarray(Wk[:, sl]).astype(bf),
            "wv": np.ascontiguousarray(Wv[:, sl]).astype(bf),
            "bqs": np.ascontiguousarray(bq[sl] * np.float32(SCALE)),
            "wo": np.ascontiguousarray(Wo[sl, :]).astype(bf),
            "maskT": mask_arr,
            "vones": vones_bf,
        })
    return in_maps


def _assemble(results, bv, Wo, bo):
    c0 = (np.asarray(bv, np.float32) @ np.asarray(Wo, np.float32)
          + np.asarray(bo, np.float32))
    out = np.empty((B, T, D), np.float32)
    for b in range(B):
        out[b] = (results[2 * b]["outT"][:, :T].T
                  + results[2 * b + 1]["outT"][:, :T].T + c0)
    return out


def kernel(hidden_states, Wq, bq, Wk, Wv, bv, Wo, bo):
    in_maps = _make_in_maps(hidden_states, Wq, bq, Wk, Wv, Wo)
    res = bass_utils.run_bass_kernel_spmd(
        _get_nc(), in_maps, core_ids=list(range(NCORES))
    )
    return _assemble(res.results, bv, Wo, bo)


# revision 32
# speedup vs baseline: 1.0148x; 1.0148x over previous
"""Chunk-causal Whisper attention (B=4, T=1500, D=1024, H=16) on 8 NeuronCores.

Sharding: core c = (batch b = c//2, head-half hh = c%2). Each core runs one
batch element with 8 of the 16 heads (512 of 1024 channels). All on-chip
tensors are kept transposed: scoresT[k,q] = K @ Q^T per head, so the softmax
reduction runs along the partition (key) axis. A ones-column appended to V
makes the PV matmul produce both the unnormalized output and the softmax
denominator; normalization broadcasts 1/denom across partitions with a gpsimd
partition_broadcast and one DVE multiply. The Wo matmul consumes the
transposed attention output directly; the host sums the two head-half
partials and adds the constant (bv @ Wo + bo) (bv folds through softmax
because the probabilities sum to 1).

All matmul operands are bf16 (PSUM accumulation stays fp32). Score tiles are
exp'd in PAIRS (one scalar-engine activation over two adjacent PSUM banks) to
amortize the activation engine's access latency; masking is a multiplicative
{0,1} bf16 mask applied to exp(scores) on the vector engine (the additive
-inf form is equivalent since exp(s+m) = exp(s)*exp(m)).

Schedule: a minimal phase A (K tb0/tb1 cc0, Q tb0 cc0, V tk0-4) unblocks
attention early; the remaining Q/K/V projection units and the per-qc output
projections are interleaved between attention groups so the tensor engine
stays fed while attention alone would be activation-bound (qc1/qc2), and
attention pairs are software-pipelined with lag 1 so PV matmuls never block
the scores stream on the exp latency. Output-projection results stream to
DRAM per (oc, qc) tile.

T is padded 1500 -> 1536 with zeros (hsT columns), so every tile is a clean
128/512 multiple. Padded keys contribute nothing (V rows and the ones-column
are zero there); padded query columns are finite garbage and dropped on the
host. The chunk-causal mask (chunk(j) <= chunk(i) or j <= i+50 with
chunk=100-frame blocks) is monotone per query, so each (128-key x 512-query)
block is fully-allowed, fully-masked (skipped), or partial.
"""

import sys

import numpy as np
import ml_dtypes

if "/opt/trn_rl_repo" not in sys.path:
    sys.path.insert(0, "/opt/trn_rl_repo")

import concourse.tile as tile  # noqa: E402
from concourse import bacc, mybir  # noqa: E402
import concourse.bass_utils as bass_utils  # noqa: E402

B, T, D = 4, 1500, 1024
H, HD = 16, 64
CHUNK, LOOK = 100, 50
TP = 1536          # padded sequence length
CH = 512           # channels per core (8 heads)
HL = 8             # heads per core
NCORES = 8
SCALE = HD ** -0.5
NEG = -30.0
QB = 512           # query block (matmul moving free dim)
KB = 128           # key block (contraction tile)
NQC = TP // QB     # 3
NKC = TP // KB     # 12
NDC = D // 128     # 8
NCC = CH // 128    # 4
NOC = D // 128     # 8 output-column chunks
F32 = mybir.dt.float32
BF16 = mybir.dt.bfloat16


def _jmax(i):
    """Largest key index query i may attend to."""
    return max((i // CHUNK) * CHUNK + CHUNK - 1, i + LOOK)


def _classify():
    """Per (qc, kc) block: 'skip' | 'full' | index into the mask tensor."""
    status = {}
    masks = []
    for qc in range(NQC):
        q0 = qc * QB
        hi = max(_jmax(i) for i in range(q0, min(q0 + QB, T)))
        lo = _jmax(q0)
        for kc in range(NKC):
            k0 = kc * KB
            if k0 > hi:
                status[(qc, kc)] = "skip"
                continue
            if k0 + KB - 1 <= lo and k0 + KB <= T:
                status[(qc, kc)] = "full"
                continue
            # multiplicative {0,1} mask applied to exp(scores)
            m = np.zeros((KB, QB), np.float32)
            for ii in range(QB):
                i = min(q0 + ii, T - 1)  # padded queries reuse the last row
                n_ok = min(min(_jmax(i), T - 1) + 1 - k0, KB)
                if n_ok > 0:
                    m[:n_ok, ii] = 1.0
            status[(qc, kc)] = len(masks)
            masks.append(m)
    return status, masks


_STATUS, _MASKS = _classify()
NPART = len(_MASKS)

# Vx ones-column values: 1.0 for real keys, 0.0 for the padded tail.
_VONES = np.zeros((128, NKC, HL), np.float32)
for _tk in range(NKC):
    for _p in range(128):
        if _tk * KB + _p < T:
            _VONES[_p, _tk, :] = 1.0


def _build():
    nc = bacc.Bacc("TRN2", target_bir_lowering=False, debug=False)
    hsT = nc.dram_tensor("hsT", [D, TP], BF16, kind="ExternalInput")[:]
    wq = nc.dram_tensor("wq", [D, CH], BF16, kind="ExternalInput")[:]
    wk = nc.dram_tensor("wk", [D, CH], BF16, kind="ExternalInput")[:]
    wv = nc.dram_tensor("wv", [D, CH], BF16, kind="ExternalInput")[:]
    bqs = nc.dram_tensor("bqs", [CH], F32, kind="ExternalInput")[:]
    wo = nc.dram_tensor("wo", [CH, D], BF16, kind="ExternalInput")[:]
    maskT = nc.dram_tensor("maskT", [NPART, KB, QB], BF16, kind="ExternalInput")[:]
    vones = nc.dram_tensor("vones", [128, NKC, HL], BF16, kind="ExternalInput")[:]
    outT = nc.dram_tensor("outT", [D, TP], F32, kind="ExternalOutput")[:]

    hsT_r = hsT.rearrange("(a p) t -> a p t", p=128)
    wq_r = wq.rearrange("(a p) c -> a p c", p=128)
    wk_r = wk.rearrange("(a p) c -> a p c", p=128)
    wv_r = wv.rearrange("(a p) c -> a p c", p=128)
    wo_r = wo.rearrange("(a p) o -> a p o", p=128)
    outT_r = outT.rearrange("(a p) t -> a p t", p=128)

    ExpF = mybir.ActivationFunctionType.Exp

    with tile.TileContext(nc) as tc, \
         tc.tile_pool(name="per", bufs=1) as per, \
         tc.tile_pool(name="ex", bufs=6) as ep, \
         tc.tile_pool(name="sm", bufs=6) as sp, \
         tc.tile_pool(name="fin", bufs=3) as fp, \
         tc.tile_pool(name="ps_p", bufs=2, space="PSUM") as ps_p, \
         tc.tile_pool(name="ps_s", bufs=2, space="PSUM") as ps_s, \
         tc.tile_pool(name="ps_pv", bufs=2, space="PSUM") as ps_pv:
        KT = per.tile([128, NCC, TP], BF16)        # K^T: [c, cc, t]
        QT = per.tile([128, NCC, TP], BF16)        # Q^T (scale+bias folded)
        Vx = per.tile([128, NKC, HL, HD + 1], BF16)  # V + ones column
        AT = per.tile([128, NCC, TP], BF16)        # normalized attnT
        hs_sb = per.tile([128, NDC, TP], BF16)
        wk_sb = per.tile([128, NDC, CH], BF16)
        wq_sb = per.tile([128, NDC, CH], BF16)
        wv_sb = per.tile([128, NDC, CH], BF16)
        wo_sb = per.tile([128, NCC, D], BF16)
        mk_sb = per.tile([128, NPART, QB], BF16)
        bq_sb = per.tile([128, NCC], F32)

        # DMA priority: feed the first projection chains (hs+wk) before
        # anything else; weights/masks needed later come later.
        # vones must land before any V-projection copy touches Vx: its strided
        # 2-byte column writes race with concurrent engine writes to adjacent
        # bytes of the same tile if the DMA is deferred.
        nc.sync.dma_start(bq_sb[:], bqs.rearrange("(a p) -> p a", p=128))
        nc.sync.dma_start(Vx[:, :, :, HD:HD + 1], vones[:, :, :, None])
        for dc in range(NDC):
            nc.sync.dma_start(hs_sb[:, dc, :], hsT_r[dc])
            nc.sync.dma_start(wk_sb[:, dc, :], wk_r[dc])
        for dc in range(NDC):
            nc.sync.dma_start(wq_sb[:, dc, :], wq_r[dc])
            nc.sync.dma_start(wv_sb[:, dc, :], wv_r[dc])
        nc.sync.dma_start(mk_sb[:], maskT.rearrange("n p q -> p n q"))
        for cc in range(NCC):
            nc.sync.dma_start(wo_sb[:, cc, :], wo_r[cc])

        # ---- projection unit emitters (one PSUM bank each) ----
        def emit_k(tb, cc, act, t0=0, t1=QB):
            ts = slice(tb * QB + t0, tb * QB + t1)
            ps = ps_p.tile([128, QB], F32, tag="p", name=f"k{tb}{cc}")
            for dc in range(NDC):
                nc.tensor.matmul(
                    ps[:, 0:t1 - t0], wk_sb[:, dc, cc * 128:(cc + 1) * 128],
                    hs_sb[:, dc, ts], start=(dc == 0), stop=(dc == NDC - 1))
            if act:
                nc.scalar.copy(KT[:, cc, ts], ps[:, 0:t1 - t0])
            else:
                nc.vector.tensor_copy(KT[:, cc, ts], ps[:, 0:t1 - t0])

        def emit_q(tb, cc, act):
            ts = slice(tb * QB, (tb + 1) * QB)
            ps = ps_p.tile([128, QB], F32, tag="p", name=f"q{tb}{cc}")
            for dc in range(NDC):
                nc.tensor.matmul(
                    ps[:], wq_sb[:, dc, cc * 128:(cc + 1) * 128],
                    hs_sb[:, dc, ts], start=(dc == 0), stop=(dc == NDC - 1))
            if act:
                nc.scalar.add(QT[:, cc, ts], ps[:], bq_sb[:, cc:cc + 1])
            else:
                nc.vector.tensor_scalar_add(
                    QT[:, cc, ts], ps[:], bq_sb[:, cc:cc + 1])

        def emit_v(tk, act):
            ps = ps_p.tile([128, CH], F32, tag="p", name=f"v{tk}")
            for dc in range(NDC):
                nc.tensor.matmul(
                    ps[:], hs_sb[:, dc, tk * KB:(tk + 1) * KB],
                    wv_sb[:, dc, :], start=(dc == 0), stop=(dc == NDC - 1))
            dst = Vx[:, tk, :, 0:HD]
            src = ps[:].rearrange("p (h d) -> p h d", d=HD)
            if act:
                nc.scalar.copy(dst, src)
            else:
                nc.vector.tensor_copy(dst, src)

        # ---- attention group: software-pipelined pairs (lag 1) ----
        def attn(h, qc):
            pb = 64 * (h % 2)
            cc = h // 2
            qs = slice(qc * QB, (qc + 1) * QB)
            kcs = [kc for kc in range(NKC) if _STATUS[(qc, kc)] != "skip"]
            pairs = [kcs[i:i + 2] for i in range(0, len(kcs), 2)]
            nk = len(kcs)
            pv = ps_pv.tile([HD + 1, QB], F32)
            state = [0]

            def emit_pv(pair_kcs, ex):
                for j, kc in enumerate(pair_kcs):
                    nc.tensor.matmul(
                        pv[:], Vx[:, kc, h, :], ex[:, j, :],
                        start=(state[0] == 0), stop=(state[0] == nk - 1))
                    state[0] += 1

            # two pairs per pipeline step: scores batch in fours before the
            # previous step's PV batch, halving PE weight-shape switches and
            # doubling the exp->PV lag slack.
            pending = []
            for i in range(0, len(pairs), 2):
                chunk = pairs[i:i + 2]
                staged = []
                for pair in chunk:
                    ss = ps_s.tile([128, 2, QB], F32)
                    for j, kc in enumerate(pair):
                        nc.tensor.matmul(
                            ss[:, j, :],
                            KT[pb:pb + 64, cc, kc * KB:(kc + 1) * KB],
                            QT[pb:pb + 64, cc, qs], start=True, stop=True)
                    ex = ep.tile([128, 2, QB], BF16, tag="e", name="e")
                    if len(pair) == 2:
                        nc.scalar.activation(ex[:], ss[:], ExpF)
                    else:
                        nc.scalar.activation(ex[:, 0, :], ss[:, 0, :], ExpF)
                    sts = [_STATUS[(qc, kc)] for kc in pair]
                    if (len(pair) == 2 and sts[0] != "full"
                            and sts[1] != "full" and sts[1] == sts[0] + 1):
                        nc.vector.tensor_mul(
                            ex[:], ex[:], mk_sb[:, sts[0]:sts[0] + 2, :])
                    else:
                        for j, st in enumerate(sts):
                            if st != "full":
                                nc.vector.tensor_mul(
                                    ex[:, j, :], ex[:, j, :], mk_sb[:, st, :])
                    staged.append((pair, ex))
                for pk, e in pending:
                    emit_pv(pk, e)
                pending = staged
            for pk, e in pending:
                emit_pv(pk, e)
            # normalize: 1/denominator broadcast across the head partitions
            dn = sp.tile([1, QB], F32, tag="dn", name="dn")
            nc.vector.tensor_copy(dn[:], pv[HD:HD + 1, :])
            rc = sp.tile([1, QB], F32, tag="recip", name="recip")
            nc.vector.reciprocal_approx_fast(rc[:], dn[:])
            bc = sp.tile([HD, QB], F32, tag="bcast", name="bcast")
            nc.gpsimd.partition_broadcast(bc[:], rc[:])
            nc.vector.tensor_mul(AT[pb:pb + 64, cc, qs], pv[0:HD, :], bc[:])

        def phase3(qc, ocs=range(NOC)):
            qs = slice(qc * QB, (qc + 1) * QB)
            for oc in ocs:
                po = ps_p.tile([128, QB], F32, tag="p", name=f"o{oc}")
                for ccc in range(NCC):
                    nc.tensor.matmul(
                        po[:], wo_sb[:, ccc, oc * 128:(oc + 1) * 128],
                        AT[:, ccc, qs], start=(ccc == 0), stop=(ccc == NCC - 1))
                fin = fp.tile([128, QB], F32, tag="fin", name="fin")
                nc.vector.tensor_copy(fin[:], po[:])
                nc.sync.dma_start(outT_r[oc][:, qs], fin[:])

        # ---- phase A: minimal head so attention can start early ----
        # K/Q epilogues on ACT, V copies on DVE, so ACT is free for the
        # first score exps.
        emit_k(0, 0, True)
        emit_k(1, 0, True)
        emit_q(0, 0, True)
        for tk in range(5):
            emit_v(tk, False)

        # Projections interleaved between attention groups (epilogues on
        # DVE).  Late-needed units (V9-11, K tb2 tail, Q tb2) are pushed
        # into the qc1 region, where attention alone leaves the tensor
        # engine idle (ACT-bound); k8 = the kc8 slice of K tb2 that qc1
        # itself needs.
        ILV = {
            (0, 0): [("k", 0, 1), ("k", 1, 1), ("q", 0, 1)],
            (0, 1): [("k", 0, 2), ("k", 1, 2), ("q", 0, 2)],
            (0, 2): [("k", 0, 3), ("k", 1, 3), ("q", 0, 3)],
            (0, 3): [("q", 1, 0), ("q", 1, 1)],
            (0, 4): [("q", 1, 2), ("q", 1, 3)],
            (0, 5): [("v", 5, None), ("v", 6, None)],
            (0, 6): [("v", 7, None), ("v", 8, None)],
            (0, 7): [("k8", 2, 0), ("k8", 2, 1), ("k8", 2, 2), ("k8", 2, 3)],
            (1, 0): [("v", 9, None), ("kr", 2, 0)],
            (1, 1): [("v", 10, None), ("kr", 2, 1)],
            (1, 2): [("v", 11, None), ("kr", 2, 2)],
            (1, 3): [("kr", 2, 3), ("q", 2, 0), ("p3", 0, (0, 2))],
            (1, 4): [("q", 2, 1), ("p3", 0, (2, 4))],
            (1, 5): [("q", 2, 2), ("p3", 0, (4, 6))],
            (1, 6): [("q", 2, 3), ("p3", 0, (6, 8))],
            (2, 0): [("p3", 1, (0, 2))],
            (2, 1): [("p3", 1, (2, 4))],
            (2, 2): [("p3", 1, (4, 6))],
            (2, 3): [("p3", 1, (6, 8))],
        }

        for qc in range(NQC):
            for h in range(HL):
                attn(h, qc)
                for kind, a, b in ILV.get((qc, h), []):
                    if kind == "k":
                        emit_k(a, b, False)
                    elif kind == "k8":
                        emit_k(a, b, False, t0=0, t1=KB)
                    elif kind == "kr":
                        emit_k(a, b, False, t0=KB, t1=QB)
                    elif kind == "q":
                        emit_q(a, b, False)
                    elif kind == "p3":
                        phase3(a, range(*b))
                    else:
                        emit_v(a, False)
            if qc == NQC - 1:
                phase3(qc)

    nc.finalize()
    return nc


_NC = None


def _get_nc():
    global _NC
    if _NC is None:
        _NC = _build()
    return _NC


def _make_in_maps(hidden_states, Wq, bq, Wk, Wv, Wo):
    hs = np.ascontiguousarray(hidden_states, np.float32)
    Wq = np.asarray(Wq, np.float32)
    Wk = np.asarray(Wk, np.float32)
    Wv = np.asarray(Wv, np.float32)
    Wo = np.asarray(Wo, np.float32)
    bq = np.asarray(bq, np.float32)

    bf = ml_dtypes.bfloat16
    mask_arr = np.ascontiguousarray(np.stack(_MASKS)).astype(bf)
    wq_s = Wq * np.float32(SCALE)
    vones_bf = _VONES.astype(bf)

    in_maps = []
    for core in range(NCORES):
        b, hh = core // 2, core % 2
        sl = slice(hh * CH, (hh + 1) * CH)
        hsT_pad = np.zeros((D, TP), np.float32)
        hsT_pad[:, :T] = hs[b].T
        in_maps.append({
            "hsT": hsT_pad.astype(bf),
            "wq": np.ascontiguousarray(wq_s[:, sl]).astype(bf),
            "wk": np.ascontiguousarray(Wk[:, sl]).astype(bf),
            "wv": np.ascontiguousarray(Wv[:, sl]).astype(bf),
            "bqs": np.ascontiguousarray(bq[sl] * np.float32(SCALE)),
            "wo": np.ascontiguousarray(Wo[sl, :]).astype(bf),
            "maskT": mask_arr,
            "vones": vones_bf,
        })
    return in_maps


def _assemble(results, bv, Wo, bo):
    c0 = (np.asarray(bv, np.float32) @ np.asarray(Wo, np.float32)
          + np.asarray(bo, np.float32))
    out = np.empty((B, T, D), np.float32)
    for b in range(B):
        out[b] = (results[2 * b]["outT"][:, :T].T
                  + results[2 * b + 1]["outT"][:, :T].T + c0)
    return out


def kernel(hidden_states, Wq, bq, Wk, Wv, bv, Wo, bo):
    in_maps = _make_in_maps(hidden_states, Wq, bq, Wk, Wv, Wo)
    res = bass_utils.run_bass_kernel_spmd(
        _get_nc(), in_maps, core_ids=list(range(NCORES))
    )
    return _assemble(res.results, bv, Wo, bo)
